# revision 1
# baseline (speedup 1.0000x reference)
"""KStepRGCN Trainium2 kernel: 8-core SPMD Bass/Tile implementation.

Sharding: nodes partitioned into 8 dst-slices (graph-partition style).
Each core aggregates messages for its dst-slice via dma_gather (bf16 rows
from a replicated node-feature table) + PE one-hot segment-sum matmuls,
then applies the per-relation basis-decomposed transforms in fp32.
Between layers the updated slices are AllGathered into the next table.
"""

import sys

sys.path.insert(0, "/opt/trn_rl_repo")

import numpy as np
import ml_dtypes

BF16 = ml_dtypes.bfloat16

# problem constants (hardcoded per harness contract)
N, E, D, R, B, K = 50000, 600000, 128, 3, 3, 3
NCORES = 8
LO_LIMIT = 32768
SEGC = 8  # chunks per gather segment; 8*128 idxs = 64 descs/SDMA lane (single_packet ceiling)


class Cfg:
    def __init__(self, n=N, e=E, ncores=NCORES):
        assert n % ncores == 0
        self.n, self.e, self.ncores = n, e, ncores
        self.ns = n // ncores                 # real nodes per slice
        self.tpc = (self.ns + 127) // 128     # col tiles per relation
        self.nsp = self.tpc * 128             # padded slice
        self.trows = ncores * self.nsp        # table rows
        self.nblk = R * self.tpc              # psum blocks per layer


def _preprocess(cfg, edge_index, edge_attr):
    """Build per-core static schedule + host tensors.

    Returns (sched, per_core list of dict, perms list).
    """
    src = np.asarray(edge_index[0], dtype=np.int64)
    dst = np.asarray(edge_index[1], dtype=np.int64)
    attr = np.asarray(edge_attr, dtype=np.int64)
    ns, nsp, tpc, nc_ = cfg.ns, cfg.nsp, cfg.tpc, cfg.ncores

    deg_total = np.bincount(dst, minlength=cfg.n)  # mean divisor (all relations)

    # --- per-core node permutation: snake-balance total degree across blocks
    perms = []
    for c in range(nc_):
        deg_local = deg_total[c * ns:(c + 1) * ns]
        order = np.argsort(-deg_local, kind="stable")
        i = np.arange(ns)
        g, o = i // tpc, i % tpc
        b = np.where(g % 2 == 0, o, tpc - 1 - o)      # snake over bins
        perm = np.empty(ns, dtype=np.int64)
        perm[order] = b * 128 + g
        perms.append(perm)

    # table row for each global node
    row_of = np.empty(cfg.n, dtype=np.int64)
    for c in range(nc_):
        row_of[c * ns:(c + 1) * ns] = c * nsp + perms[c]

    lo_lim = min(LO_LIMIT, cfg.trows)
    hi_rows = cfg.trows - lo_lim

    # --- per-core edge bucketing by (stream, block), fully vectorized
    core_of = dst // ns
    streams_pc = []   # per core: (lo=(row, bl, colw, rank), hi=(...))
    max_lo = 1
    max_hi = 0
    for c in range(nc_):
        m = core_of == c
        s_c, v_c, r_c = src[m], dst[m] - c * ns, attr[m]
        pos = perms[c][v_c]
        bl = r_c * tpc + pos // 128
        colw = pos % 128
        row = row_of[s_c]
        is_lo = row < lo_lim
        parts = []
        for sel, base in ((is_lo, 0), (~is_lo, lo_lim)):
            blv, rv, cv = bl[sel], row[sel] - base, colw[sel]
            order = np.argsort(blv, kind="stable")
            blv, rv, cv = blv[order], rv[order], cv[order]
            cnt = np.bincount(blv, minlength=cfg.nblk)
            start = np.concatenate(([0], np.cumsum(cnt)))
            rank = np.arange(len(blv)) - start[blv]
            parts.append((rv, blv, cv, rank, cnt))
        streams_pc.append(parts)
        max_lo = max(max_lo, int(np.ceil(parts[0][4].max() / 128)) if parts[0][4].size else 1)
        max_hi = max(max_hi, int(np.ceil(parts[1][4].max() / 128)) if parts[1][4].size and parts[1][4].max() else 0)

    cpb_lo, cpb_hi = max(1, max_lo), max_hi
    nlo_chunks = cfg.nblk * cpb_lo
    nhi_chunks = cfg.nblk * cpb_hi
    nlo_seg = (nlo_chunks + SEGC - 1) // SEGC
    nhi_seg = (nhi_chunks + SEGC - 1) // SEGC if nhi_chunks else 0

    def build_stream(part, cpb, nseg):
        """idx [128, nseg*SEGC*8] i16 ; S [128, nseg*SEGC*128] bf16"""
        rv, blv, cv, rank, _ = part
        tot = nseg * SEGC
        idx_flat = np.zeros(tot * 128, dtype=np.int16)
        S = np.zeros((128, tot * 128), dtype=np.float32)
        if len(rv):
            ch = blv * cpb + rank // 128
            epos = rank % 128
            idx_flat[ch * 128 + epos] = rv.astype(np.int16)
            S[epos, ch * 128 + cv] = 1.0
        idx_w = np.tile(
            idx_flat.reshape(nseg, SEGC * 8, 16).transpose(0, 2, 1)
            .reshape(nseg, 16, SEGC * 8).transpose(1, 0, 2).reshape(16, tot * 8),
            (8, 1)).astype(np.int16)
        return idx_w, S.astype(BF16)

    per_core = []
    for c in range(nc_):
        lo_p, hi_p = streams_pc[c]
        il, sl = build_stream(lo_p, cpb_lo, nlo_seg)
        if nhi_seg:
            ih, sh = build_stream(hi_p, cpb_hi, nhi_seg)
        else:
            ih = np.zeros((128, SEGC * 8), dtype=np.int16)
            sh = np.zeros((128, SEGC * 128), dtype=BF16)
        per_core.append(dict(idx_lo=il, S_lo=sl, idx_hi=ih, S_hi=sh))

    sched = dict(cpb_lo=cpb_lo, cpb_hi=cpb_hi, nlo_seg=nlo_seg, nhi_seg=nhi_seg,
                 lo_lim=lo_lim, hi_rows=hi_rows)
    inv_cnt = 1.0 / np.maximum(deg_total, 1).astype(np.float32)
    return sched, per_core, perms, inv_cnt


def _build_program(cfg, sched, k_layers=K, prelu_a=0.25):
    """Emit the Bass program. Returns (nc, names of IO tensors)."""
    from concourse import bacc, mybir
    import concourse.tile as tile

    f32, bf16, i16 = mybir.dt.float32, mybir.dt.bfloat16, mybir.dt.int16
    Alu = mybir.AluOpType
    tpc, nsp, nblk, trows = cfg.tpc, cfg.nsp, cfg.nblk, cfg.trows
    cpb_lo, cpb_hi = sched["cpb_lo"], sched["cpb_hi"]
    nlo_seg, nhi_seg = sched["nlo_seg"], sched["nhi_seg"]
    lo_lim, hi_rows = sched["lo_lim"], sched["hi_rows"]

    nc = bacc.Bacc("TRN2", target_bir_lowering=False, debug=False,
                   num_devices=cfg.ncores)

    # --- IO tensors
    x_table = nc.dram_tensor("x_table", [trows, D], bf16, kind="ExternalInput")
    x_own = nc.dram_tensor("x_own", [128, nsp], f32, kind="ExternalInput")
    w_sw = nc.dram_tensor("w_sw", [128, k_layers * R * D], f32, kind="ExternalInput")
    root_sw = nc.dram_tensor("root_sw", [128, k_layers * D], f32, kind="ExternalInput")
    bias_in = nc.dram_tensor("bias_in", [1, k_layers * D], f32, kind="ExternalInput")
    ident_in = nc.dram_tensor("ident_in", [128, 128], f32, kind="ExternalInput")
    invc_in = nc.dram_tensor("invc_in", [128, tpc], f32, kind="ExternalInput")
    idx_lo_in = nc.dram_tensor("idx_lo", [128, max(1, nlo_seg) * SEGC * 8], i16,
                               kind="ExternalInput")
    s_lo_in = nc.dram_tensor("s_lo", [128, max(1, nlo_seg) * SEGC * 128], bf16,
                             kind="ExternalInput")
    idx_hi_in = nc.dram_tensor("idx_hi", [128, max(1, nhi_seg) * SEGC * 8], i16,
                               kind="ExternalInput")
    s_hi_in = nc.dram_tensor("s_hi", [128, max(1, nhi_seg) * SEGC * 128], bf16,
                             kind="ExternalInput")
    out_own = nc.dram_tensor("out_own", [nsp, D], f32, kind="ExternalOutput")


    # internal tables for AllGather
    ag_in = nc.dram_tensor("ag_in", [nsp, D], bf16, kind="Internal")
    tables = [x_table]
    for i in range(k_layers - 1):
        tables.append(nc.dram_tensor(f"table{i + 1}", [trows, D], bf16,
                                     kind="Internal", addr_space="Shared"))

    rg = [list(range(cfg.ncores))]

    from contextlib import ExitStack

    with tile.TileContext(nc) as tc, ExitStack() as ctx:
        const = ctx.enter_context(tc.tile_pool(name="const", bufs=1))
        w_t = const.tile([128, k_layers * R * D], f32, tag="w")
        root_t = const.tile([128, k_layers * D], f32, tag="root")
        bias_t = const.tile([1, k_layers * D], f32, tag="bias")
        ones_t = const.tile([1, 128], f32, tag="ones")
        ident_t = const.tile([128, 128], f32, tag="ident")
        invc_t = const.tile([128, tpc], f32, tag="invc")
        h_own = const.tile([128, nsp], f32, tag="h_own")
        a_T = const.tile([128, nblk * 128], f32, tag="a_T")
        idx_lo_t = const.tile([128, max(1, nlo_seg) * SEGC * 8], i16, tag="ixl")
        idx_hi_t = const.tile([128, max(1, nhi_seg) * SEGC * 8], i16, tag="ixh")
        hbf = const.tile([128, nsp], bf16, tag="hbf")

        nc.sync.dma_start(w_t[:], w_sw.ap())
        nc.sync.dma_start(root_t[:], root_sw.ap())
        nc.sync.dma_start(bias_t[:], bias_in.ap())
        nc.sync.dma_start(ident_t[:], ident_in.ap())
        nc.sync.dma_start(invc_t[:], invc_in.ap())
        nc.sync.dma_start(h_own[:], x_own.ap())
        nc.sync.dma_start(idx_lo_t[:], idx_lo_in.ap())
        if nhi_seg:
            nc.sync.dma_start(idx_hi_t[:], idx_hi_in.ap())
        nc.vector.memset(ones_t[:], 1.0)

        msg_lo = ctx.enter_context(tc.tile_pool(name="msg_lo", bufs=2))
        msg_hi = ctx.enter_context(tc.tile_pool(name="msg_hi", bufs=2))
        sp_lo = ctx.enter_context(tc.tile_pool(name="sp_lo", bufs=2))
        sp_hi = ctx.enter_context(tc.tile_pool(name="sp_hi", bufs=2))
        pblk = ctx.enter_context(tc.tile_pool(name="pblk", bufs=2, space="PSUM"))
        pout = ctx.enter_context(tc.tile_pool(name="pout", bufs=2, space="PSUM"))
        proot = ctx.enter_context(tc.tile_pool(name="proot", bufs=2, space="PSUM"))
        ptr_p = ctx.enter_context(tc.tile_pool(name="ptr", bufs=2, space="PSUM"))
        hT_pool = ctx.enter_context(tc.tile_pool(name="hT", bufs=2))
        tmp_pool = ctx.enter_context(tc.tile_pool(name="tmp", bufs=2))

        # staged SWDGE path: auto-trigger dma_gather crashes this runtime,
        # prepare_only + trigger_dma works. Persistent sems, cumulative targets.
        prep_sem = ctx.enter_context(nc.semaphore())
        dma_sem = ctx.enter_context(nc.semaphore())
        gcount = [0]

        def emit_gather(mt, in_ap, idxs_ap):
            gcount[0] += 1
            g = gcount[0]
            with tc.tile_critical():
                nc.gpsimd.dma_gather(
                    out_ap=mt[:], in_ap=in_ap, idxs_ap=idxs_ap,
                    num_idxs=SEGC * 128, num_idxs_reg=SEGC * 128, elem_size=D,
                    prepare_only=True, sem=dma_sem).then_inc(prep_sem, 1)
                nc.gpsimd.wait_ge(prep_sem, g)
                nc.gpsimd.trigger_dma(count=1)
                nc.gpsimd.wait_ge(dma_sem, 16 * g)

        # gather segments must be emitted in consumption order (blocks read
        # lo and hi tiles interleaved; pools have finite bufs)
        seg_order = sorted(
            [("lo", s, s * SEGC // cpb_lo) for s in range(nlo_seg)]
            + ([("hi", s, s * SEGC // cpb_hi) for s in range(nhi_seg)]
               if nhi_seg else []),
            key=lambda t: (t[2], t[0] == "hi"))

        for k in range(k_layers):
            table = tables[k]
            # ---- gather segments + S loads, in consumption order
            lo_tiles, lo_S = {}, {}
            hi_tiles, hi_S = {}, {}
            for stream, s, _ in seg_order:
                if stream == "lo":
                    mt = msg_lo.tile([128, SEGC, D], bf16, tag="m")
                    emit_gather(mt, table.ap()[0:lo_lim, :],
                                idx_lo_t[:, s * SEGC * 8:(s + 1) * SEGC * 8])
                    st = sp_lo.tile([128, SEGC * 128], bf16, tag="s")
                    nc.sync.dma_start(st[:], s_lo_in.ap()[:, s * SEGC * 128:(s + 1) * SEGC * 128])
                    lo_tiles[s], lo_S[s] = mt, st
                else:
                    mt = msg_hi.tile([128, SEGC, D], bf16, tag="m")
                    emit_gather(mt, table.ap()[lo_lim:trows, :],
                                idx_hi_t[:, s * SEGC * 8:(s + 1) * SEGC * 8])
                    st = sp_hi.tile([128, SEGC * 128], bf16, tag="s")
                    nc.sync.dma_start(st[:], s_hi_in.ap()[:, s * SEGC * 128:(s + 1) * SEGC * 128])
                    hi_tiles[s], hi_S[s] = mt, st

            # ---- segment-sum into A^T blocks
            n_mm = cpb_lo + (cpb_hi if nhi_seg else 0)
            for bl in range(nblk):
                pb = pblk.tile([128, 128], f32, tag="pb")
                mm = 0
                for j in range(cpb_lo):
                    L = bl * cpb_lo + j
                    s, pos = divmod(L, SEGC)
                    nc.tensor.matmul(pb[:], lhsT=lo_tiles[s][:, pos, :],
                                     rhs=lo_S[s][:, pos * 128:(pos + 1) * 128],
                                     start=(mm == 0), stop=(mm == n_mm - 1))
                    mm += 1
                if nhi_seg:
                    for j in range(cpb_hi):
                        L = bl * cpb_hi + j
                        s, pos = divmod(L, SEGC)
                        nc.tensor.matmul(pb[:], lhsT=hi_tiles[s][:, pos, :],
                                         rhs=hi_S[s][:, pos * 128:(pos + 1) * 128],
                                         start=(mm == 0), stop=(mm == n_mm - 1))
                        mm += 1
                nc.vector.tensor_copy(a_T[:, bl * 128:(bl + 1) * 128], pb[:])

            # ---- transform per col-tile
            for t in range(tpc):
                po = pout.tile([128, 128], f32, tag="po")
                pr = proot.tile([128, 128], f32, tag="pr")
                pt = ptr_p.tile([128, 128], f32, tag="pt")
                for r in range(R):
                    bl = r * tpc + t
                    nc.tensor.matmul(po[:], lhsT=a_T[:, bl * 128:(bl + 1) * 128],
                                     rhs=w_t[:, (k * R + r) * D:(k * R + r + 1) * D],
                                     start=(r == 0), stop=(r == R - 1))
                nc.tensor.transpose(pt[:], h_own[:, t * 128:(t + 1) * 128], ident_t[:])
                hT = hT_pool.tile([128, 128], f32, tag="h")
                nc.vector.tensor_copy(hT[:], pt[:])
                nc.tensor.matmul(pr[:], lhsT=hT[:], rhs=root_t[:, k * D:(k + 1) * D],
                                 start=True, stop=False)
                nc.tensor.matmul(pr[:], lhsT=ones_t[:], rhs=bias_t[:, k * D:(k + 1) * D],
                                 start=False, stop=True)
                tt = tmp_pool.tile([128, 128], f32, tag="t")
                # hw: only one PSUM operand per DVE op -> two steps
                nc.vector.tensor_scalar(tt[:], po[:], invc_t[:, t:t + 1], None,
                                        Alu.mult)
                dst_sl = h_own[:, t * 128:(t + 1) * 128]
                if k < k_layers - 1:
                    pre = tmp_pool.tile([128, 128], f32, tag="t2")
                    nc.vector.tensor_add(pre[:], tt[:], pr[:])
                    nc.vector.scalar_tensor_tensor(dst_sl, pre[:], prelu_a, pre[:],
                                                   Alu.mult, Alu.max)
                else:
                    nc.vector.tensor_add(dst_sl, tt[:], pr[:])

            # ---- export: cast + AllGather (not after last layer)
            if k < k_layers - 1:
                nc.vector.tensor_copy(hbf[:], h_own[:])
                nc.sync.dma_start(
                    ag_in.ap().rearrange("(t p) f -> p t f", p=128),
                    hbf[:].rearrange("p (t f) -> p t f", f=D))
                nc.gpsimd.collective_compute(
                    "AllGather", Alu.bypass, replica_groups=rg,
                    ins=[ag_in.ap()], outs=[tables[k + 1].ap()])

        nc.sync.dma_start(out_own.ap().rearrange("(t p) f -> p t f", p=128),
                          h_own[:].rearrange("p (t f) -> p t f", f=D))

    nc.compile()
    return nc


def _host_tensors(cfg, sched, per_core, perms, inv_cnt, x, basis, att, root, bias,
                  k_layers=K):
    """Build in_maps for all cores."""
    ns, nsp, tpc = cfg.ns, cfg.nsp, cfg.tpc
    # relation weights W[k] = einsum('rb,bio->rio', att[k], basis[k])
    W = np.einsum("krb,kbio->krio", att.astype(np.float32),
                  basis.astype(np.float32))[:k_layers]  # [k,R,D,D]
    root = root[:k_layers]
    bias = bias[:k_layers]
    w_sw = np.ascontiguousarray(
        W.transpose(2, 0, 1, 3).reshape(D, k_layers * R * D)).astype(np.float32)
    root_sw = np.ascontiguousarray(
        root.transpose(1, 0, 2).reshape(D, k_layers * D)).astype(np.float32)
    bias_in = bias.reshape(1, k_layers * D).astype(np.float32)
    ident = np.eye(128, dtype=np.float32)

    # global bf16 table [trows, D]
    table = np.zeros((cfg.trows, D), dtype=BF16)
    for c in range(cfg.ncores):
        sl = x[c * ns:(c + 1) * ns].astype(BF16)
        rowpos = c * nsp + perms[c]
        table[rowpos] = sl

    in_maps = []
    for c in range(cfg.ncores):
        x_own = np.zeros((128, nsp), dtype=np.float32)
        invc = np.ones((128, tpc), dtype=np.float32)
        inv_perm = np.empty(nsp, dtype=np.int64)
        inv_perm.fill(-1)
        for v in range(ns):
            inv_perm[perms[c][v]] = v
        for t in range(tpc):
            for p in range(128):
                v = inv_perm[t * 128 + p]
                if v >= 0:
                    x_own[p, t * 128:(t + 1) * 128] = x[c * ns + v]
                    invc[p, t] = inv_cnt[c * ns + v]
        pc = per_core[c]
        in_maps.append(dict(
            x_table=table, x_own=x_own, w_sw=w_sw, root_sw=root_sw,
            bias_in=bias_in, ident_in=ident, invc_in=invc,
            idx_lo=pc["idx_lo"], s_lo=np.ascontiguousarray(pc["S_lo"]),
            idx_hi=pc["idx_hi"], s_hi=np.ascontiguousarray(pc["S_hi"])))
    return in_maps


def _run(cfg, x, edge_index, edge_attr, basis, att, root, bias, prelu_a,
         k_layers=K, trace=False):
    from concourse.bass_utils import run_bass_kernel_spmd

    sched, per_core, perms, inv_cnt = _preprocess(cfg, edge_index, edge_attr)
    nc = _build_program(cfg, sched, k_layers,
                        float(np.asarray(prelu_a).ravel()[0]))
    in_maps = _host_tensors(cfg, sched, per_core, perms, inv_cnt,
                            np.asarray(x, dtype=np.float32),
                            np.asarray(basis), np.asarray(att),
                            np.asarray(root), np.asarray(bias), k_layers)
    res = run_bass_kernel_spmd(nc, in_maps, core_ids=list(range(cfg.ncores)),
                               trace=trace)
    out = np.empty((cfg.n, D), dtype=np.float32)
    for c in range(cfg.ncores):
        rows = res.results[c]["out_own"]  # [nsp, D] permuted
        out[c * cfg.ns:(c + 1) * cfg.ns] = rows[perms[c]]
    return out, res


def kernel(x, edge_index, edge_attr, basis, att, root, bias, prelu_a):
    cfg = Cfg()
    out, _ = _run(cfg, x, edge_index, edge_attr, basis, att, root, bias, prelu_a)
    return out



# revision 5
# speedup vs baseline: 643.3018x; 643.3018x over previous
"""KStepRGCN Trainium2 kernel: 8-core SPMD Bass/Tile implementation (v2).

Sharding: nodes partitioned into 8 dst-slices (graph-partition style).
Per layer, each core gathers the bf16 feature rows of its edges' sources
from a replicated HBM table (SWDGE dma_gather, pipelined with
consumer-side completion waits), segment-sums them into per-(tile,
relation) A^T blocks via PE one-hot matmuls whose one-hot S matrices are
generated on-chip (DVE iota==cv compare, batched per gather segment),
then applies the basis-decomposed relation transforms. The root term is
folded in as a 4th "relation": a PE transpose of the local h tile
against diag(cnt) so the subsequent 1/cnt mean scaling cancels. Between
layers the updated bf16 slices are AllGathered into the next table.
"""

import sys

sys.path.insert(0, "/opt/trn_rl_repo")

import numpy as np
import ml_dtypes

BF16 = ml_dtypes.bfloat16

# problem constants (hardcoded per harness contract)
N, E, D, R, B, K = 50000, 600000, 128, 3, 3, 3
NCORES = 8
LO_LIMIT = 32768
SEGC_LO, SEGC_HI = 32, 16  # chunks per gather segment, per index stream
NSELF = R  # slot index of the self/root pseudo-relation (blocks per tile = R+1)


class Cfg:
    def __init__(self, n=N, e=E, ncores=NCORES):
        assert n % ncores == 0
        self.n, self.e, self.ncores = n, e, ncores
        self.ns = n // ncores                 # real nodes per slice
        self.tpc = (self.ns + 127) // 128     # dst col tiles per core
        self.nsp = self.tpc * 128             # padded slice
        self.trows = ncores * self.nsp        # table rows
        self.nblk = R * self.tpc              # (tile, relation) blocks
        self.lo_lim = min(LO_LIMIT, self.trows)


def _pack_idx(rows_flat, nseg, segc):
    """rows_flat [nseg*segc*128] -> [128, nseg*segc*8] i16 (16p wrap, 8x rep)."""
    tot = nseg * segc
    arr = rows_flat.reshape(nseg, segc * 8, 16).transpose(0, 2, 1)
    arr = arr.reshape(nseg, 16, segc * 8).transpose(1, 0, 2).reshape(16, tot * 8)
    return np.tile(arr, (8, 1)).astype(np.int16)


def _preprocess(cfg, edge_index, edge_attr):
    """Shared (cross-core uniform) schedule + per-core gather/scatter data."""
    src = np.asarray(edge_index[0], dtype=np.int64)
    dst = np.asarray(edge_index[1], dtype=np.int64)
    attr = np.asarray(edge_attr, dtype=np.int64)
    ns, nsp, tpc, nblk, nc_ = cfg.ns, cfg.nsp, cfg.tpc, cfg.nblk, cfg.ncores
    lo_lim = cfg.lo_lim

    row = (src // ns) * nsp + (src % ns)
    reg = (row >= lo_lim).astype(np.int64)
    rowrel = row - reg * lo_lim
    core = dst // ns
    v = dst - core * ns
    bl = (v >> 7) * R + attr
    cvv = v & 127
    key = bl * 2 + reg

    counts = np.zeros((nc_, nblk, 2), dtype=np.int64)
    for c in range(nc_):
        counts[c] = np.bincount(key[core == c], minlength=nblk * 2).reshape(nblk, 2)
    cpb = -(-counts.max(axis=0) // 128)  # [nblk, 2] chunks per block (shared)

    lo_start = np.concatenate(([0], np.cumsum(cpb[:, 0])))
    hi_start = np.concatenate(([0], np.cumsum(cpb[:, 1])))
    CL, CH = int(lo_start[-1]), int(hi_start[-1])
    nlo_seg = max(1, -(-CL // SEGC_LO))
    nhi_seg = -(-CH // SEGC_HI) if CH else 0
    CLpad = nlo_seg * SEGC_LO
    CHpad = max(1, nhi_seg) * SEGC_HI

    per_core = []
    for c in range(nc_):
        m = core == c
        kb, rr, cvc = key[m], rowrel[m], cvv[m]
        o = np.argsort(kb, kind="stable")
        kb, rr, cvc = kb[o], rr[o], cvc[o]
        cnt_k = np.bincount(kb, minlength=nblk * 2)
        start_k = np.concatenate(([0], np.cumsum(cnt_k)))
        rank = np.arange(len(kb)) - start_k[kb]
        blv, regv = kb // 2, kb % 2
        stream_chunk = np.where(regv == 0, lo_start[blv], hi_start[blv]) + rank // 128
        slot = rank % 128
        rows_lo = np.zeros(CLpad * 128, dtype=np.int64)
        cv_lo = np.full(CLpad * 128, 255, dtype=np.int64)
        rows_hi = np.zeros(CHpad * 128, dtype=np.int64)
        cv_hi = np.full(CHpad * 128, 255, dtype=np.int64)
        is_lo = regv == 0
        pos_lo = stream_chunk[is_lo] * 128 + slot[is_lo]
        rows_lo[pos_lo] = rr[is_lo]
        cv_lo[pos_lo] = cvc[is_lo]
        pos_hi = stream_chunk[~is_lo] * 128 + slot[~is_lo]
        rows_hi[pos_hi] = rr[~is_lo]
        cv_hi[pos_hi] = cvc[~is_lo]
        per_core.append(dict(
            idx_lo=_pack_idx(rows_lo, nlo_seg, SEGC_LO),
            cv_lo=np.ascontiguousarray(cv_lo.reshape(CLpad, 128).T).astype(BF16),
            idx_hi=_pack_idx(rows_hi, max(1, nhi_seg), SEGC_HI),
            cv_hi=np.ascontiguousarray(cv_hi.reshape(CHpad, 128).T).astype(BF16),
        ))

    sched = dict(cpb=cpb, lo_start=lo_start, hi_start=hi_start,
                 nlo_seg=nlo_seg, nhi_seg=nhi_seg, lo_lim=lo_lim)
    return sched, per_core


def _build_program(cfg, sched, k_layers=K, prelu_a=0.25):
    from concourse import bacc, mybir
    import concourse.tile as tile
    from contextlib import ExitStack

    f32, bf16, i16 = mybir.dt.float32, mybir.dt.bfloat16, mybir.dt.int16
    Alu = mybir.AluOpType
    tpc, nsp, trows, lo_lim = cfg.tpc, cfg.nsp, cfg.trows, sched["lo_lim"]
    cpb, lo_start, hi_start = sched["cpb"], sched["lo_start"], sched["hi_start"]
    nlo_seg, nhi_seg = sched["nlo_seg"], sched["nhi_seg"]
    NR = R + 1  # relations + self

    nc = bacc.Bacc("TRN2", target_bir_lowering=False, debug=False,
                   num_devices=cfg.ncores, dynamic_dma_scratch_size=32768)

    x0_in = nc.dram_tensor("x0", [128, nsp], bf16, kind="ExternalInput")
    dcs_in = nc.dram_tensor("dcs", [128, nsp], bf16, kind="ExternalInput")
    cnt_in = nc.dram_tensor("cnt_row", [1, nsp], bf16, kind="ExternalInput")
    w_in = nc.dram_tensor("w_sw", [128, k_layers * NR * D], bf16,
                          kind="ExternalInput")
    bias_in = nc.dram_tensor("bias_in", [1, k_layers * D], bf16,
                             kind="ExternalInput")
    invc_in = nc.dram_tensor("invc_in", [128, tpc], f32, kind="ExternalInput")
    iota_in = nc.dram_tensor("iota_in", [128, 128], bf16, kind="ExternalInput")
    idx_lo_in = nc.dram_tensor("idx_lo", [128, nlo_seg * SEGC_LO * 8], i16,
                               kind="ExternalInput")
    cv_lo_in = nc.dram_tensor("cv_lo", [128, nlo_seg * SEGC_LO], bf16,
                              kind="ExternalInput")
    idx_hi_in = nc.dram_tensor("idx_hi", [128, max(1, nhi_seg) * SEGC_HI * 8],
                               i16, kind="ExternalInput")
    cv_hi_in = nc.dram_tensor("cv_hi", [128, max(1, nhi_seg) * SEGC_HI], bf16,
                              kind="ExternalInput")
    out_own = nc.dram_tensor("out_own", [nsp, D], f32, kind="ExternalOutput")

    ag_in = nc.dram_tensor("ag_in", [nsp, D], bf16, kind="Internal")
    tables = [nc.dram_tensor(f"table{i}", [trows, D], bf16, kind="Internal",
                             addr_space="Shared") for i in range(k_layers)]
    rg = [list(range(cfg.ncores))]

    # per-tile chunk plan: (stream, seg, pos, r) in consumption order
    tiles_plan = []
    for t in range(tpc):
        chunks = []
        for r in range(R):
            b = t * R + r
            for j in range(int(cpb[b, 0])):
                cid = int(lo_start[b]) + j
                chunks.append(("lo", cid // SEGC_LO, cid % SEGC_LO, r))
            for j in range(int(cpb[b, 1])):
                cid = int(hi_start[b]) + j
                chunks.append(("hi", cid // SEGC_HI, cid % SEGC_HI, r))
        tiles_plan.append(chunks)

    from contextlib import ExitStack

    with tile.TileContext(nc) as tc, ExitStack() as ctx:
        const = ctx.enter_context(tc.tile_pool(name="const", bufs=1))
        hbf = const.tile([128, nsp], bf16, tag="hbf")
        dcs_t = const.tile([128, nsp], bf16, tag="dcs")
        cnt_t = const.tile([1, nsp], bf16, tag="cnt")
        w_t = const.tile([128, k_layers * NR * D], bf16, tag="w")
        bias_t = const.tile([1, k_layers * D], bf16, tag="bias")
        invc_t = const.tile([128, tpc], f32, tag="invc")
        iota_t = const.tile([128, 128], bf16, tag="iota")
        h_own = const.tile([128, nsp], f32, tag="h_own")
        idx_lo_t = const.tile([128, nlo_seg * SEGC_LO * 8], i16, tag="ixl")
        cv_lo_t = const.tile([128, nlo_seg * SEGC_LO], bf16, tag="cvl")
        idx_hi_t = const.tile([128, max(1, nhi_seg) * SEGC_HI * 8], i16,
                              tag="ixh")
        cv_hi_t = const.tile([128, max(1, nhi_seg) * SEGC_HI], bf16, tag="cvh")

        nc.sync.dma_start(hbf[:], x0_in.ap())
        nc.sync.dma_start(dcs_t[:], dcs_in.ap())
        nc.sync.dma_start(cnt_t[:], cnt_in.ap())
        nc.sync.dma_start(w_t[:], w_in.ap())
        nc.sync.dma_start(bias_t[:], bias_in.ap())
        nc.sync.dma_start(invc_t[:], invc_in.ap())
        nc.sync.dma_start(iota_t[:], iota_in.ap())
        nc.sync.dma_start(idx_lo_t[:], idx_lo_in.ap())
        nc.sync.dma_start(cv_lo_t[:], cv_lo_in.ap())
        if nhi_seg:
            nc.sync.dma_start(idx_hi_t[:], idx_hi_in.ap())
            nc.sync.dma_start(cv_hi_t[:], cv_hi_in.ap())

        msg_lo = ctx.enter_context(tc.tile_pool(name="msg_lo", bufs=2))
        msg_hi = ctx.enter_context(tc.tile_pool(name="msg_hi", bufs=2))
        sp_lo = ctx.enter_context(tc.tile_pool(name="sp_lo", bufs=2))
        sp_hi = ctx.enter_context(tc.tile_pool(name="sp_hi", bufs=2))
        a_pool = ctx.enter_context(tc.tile_pool(name="aT", bufs=3))
        tmp_pool = ctx.enter_context(tc.tile_pool(name="tmp", bufs=2))
        pblk = ctx.enter_context(tc.tile_pool(name="pblk", bufs=2, space="PSUM"))
        pout = ctx.enter_context(tc.tile_pool(name="pout", bufs=2, space="PSUM"))

        prep_sem = ctx.enter_context(nc.semaphore())
        dma_sem = ctx.enter_context(nc.semaphore())
        gcount = [0]

        segcfg = dict(
            lo=(msg_lo, sp_lo, idx_lo_t, cv_lo_t, SEGC_LO, 0, lo_lim),
            hi=(msg_hi, sp_hi, idx_hi_t, cv_hi_t, SEGC_HI, lo_lim, trows))

        for k in range(k_layers):
            # ---- export h -> table[k] (AllGather)
            nc.sync.dma_start(
                ag_in.ap().rearrange("(t p) f -> p t f", p=128),
                hbf[:].rearrange("p (t f) -> p t f", f=D))
            nc.gpsimd.collective_compute(
                "AllGather", Alu.bypass, replica_groups=rg,
                ins=[ag_in.ap()], outs=[tables[k].ap()])
            table = tables[k]

            seg_tiles = {}   # (stream, seg) -> (mt, st, g)

            def emit_seg(stream, s, k=k, table=table, seg_tiles=seg_tiles):
                if (stream, s) in seg_tiles:
                    return
                mpool, spool, idx_t, cv_t, segc, r0, r1 = segcfg[stream]
                mt = mpool.tile([128, segc, D], bf16, tag="m")
                gcount[0] += 1
                g = gcount[0]
                with tc.tile_critical():
                    nc.gpsimd.dma_gather(
                        out_ap=mt[:], in_ap=table.ap()[r0:r1, :],
                        idxs_ap=idx_t[:, s * segc * 8:(s + 1) * segc * 8],
                        num_idxs=segc * 128, num_idxs_reg=segc * 128,
                        elem_size=D, prepare_only=True, sem=dma_sem,
                        single_packet=False).then_inc(prep_sem, 1)
                    nc.gpsimd.wait_ge(prep_sem, g)
                    nc.gpsimd.trigger_dma(count=1)
                st = spool.tile([128, segc * 128], bf16, tag="s")
                st_v = st[:].rearrange("p (c f) -> p c f", f=128)
                io_b = iota_t[:].rearrange("p (o f) -> p o f", o=1).broadcast_to(
                    [128, segc, 128])
                cv_b = cv_t[:, s * segc:(s + 1) * segc].rearrange(
                    "p (c o) -> p c o", o=1).broadcast_to([128, segc, 128])
                nc.vector.tensor_tensor(st_v, io_b, cv_b, Alu.is_equal)
                seg_tiles[(stream, s)] = (mt, st, g)

            for t in range(tpc):
                chunks = tiles_plan[t]
                for stream, s, pos, r in chunks:
                    emit_seg(stream, s)
                tsl = slice(t * 128, (t + 1) * 128)
                pb = pblk.tile([128, NR * 128], f32, tag="pb")
                nprev = {}
                ntot = {r: sum(1 for c in chunks if c[3] == r) for r in range(R)}
                gmax = max((seg_tiles[(st_, s_)][2] for st_, s_, _, _ in chunks),
                           default=0)
                with tc.tile_critical():
                    if chunks:
                        nc.tensor.wait_ge(dma_sem, 16 * gmax)
                    for stream, s, pos, r in chunks:
                        mt, st, _ = seg_tiles[(stream, s)]
                        done = nprev.get(r, 0)
                        nc.tensor.matmul(
                            pb[:, r * 128:(r + 1) * 128], lhsT=mt[:, pos, :],
                            rhs=st[:, pos * 128:(pos + 1) * 128],
                            start=(done == 0), stop=(done == ntot[r] - 1))
                        nprev[r] = done + 1
                    # self/root block: transpose local h tile against diag(cnt)
                    nc.tensor.matmul(pb[:, NSELF * 128:(NSELF + 1) * 128],
                                     lhsT=hbf[:, tsl], rhs=dcs_t[:, tsl],
                                     start=True, stop=True)
                aT = a_pool.tile([128, NR * 128], bf16, tag="a")
                nc.scalar.copy(aT[:], pb[:])

                # ---- transform: po[n,f'] = sum_r A_r W_r + cnt*(h root + bias)
                gpos = t % 4
                if gpos == 0:
                    pout_g = pout.tile([128, 512], f32, tag="po")
                    g0 = t
                po = pout_g[:, gpos * 128:(gpos + 1) * 128]
                first = True
                for r in range(R):
                    if ntot[r]:
                        nc.tensor.matmul(
                            po, lhsT=aT[:, r * 128:(r + 1) * 128],
                            rhs=w_t[:, (k * NR + r) * D:(k * NR + r + 1) * D],
                            start=first, stop=False)
                        first = False
                nc.tensor.matmul(
                    po, lhsT=aT[:, NSELF * 128:(NSELF + 1) * 128],
                    rhs=w_t[:, (k * NR + NSELF) * D:(k * NR + NSELF + 1) * D],
                    start=first, stop=False)
                nc.tensor.matmul(po, lhsT=cnt_t[:, tsl],
                                 rhs=bias_t[:, k * D:(k + 1) * D],
                                 start=False, stop=True)

                if gpos == 3 or t == tpc - 1:
                    ngr = gpos + 1
                    used = ngr * 128
                    po_v = pout_g[:, :used].rearrange("p (c f) -> p c f", f=128)
                    invc_b = invc_t[:, g0:g0 + ngr].rearrange(
                        "p (c o) -> p c o", o=1).broadcast_to([128, ngr, 128])
                    base = g0 * 128
                    if k < k_layers - 1:
                        tmp = tmp_pool.tile([128, 512], f32, tag="t")
                        tmp_v = tmp[:, :used].rearrange("p (c f) -> p c f", f=128)
                        nc.vector.tensor_tensor(tmp_v, po_v, invc_b, Alu.mult)
                        nc.vector.scalar_tensor_tensor(
                            hbf[:, base:base + used], tmp[:, :used], prelu_a,
                            tmp[:, :used], Alu.mult, Alu.max)
                    else:
                        ho_v = h_own[:, base:base + used].rearrange(
                            "p (c f) -> p c f", f=128)
                        nc.vector.tensor_tensor(ho_v, po_v, invc_b, Alu.mult)

        nc.sync.dma_start(out_own.ap().rearrange("(t p) f -> p t f", p=128),
                          h_own[:].rearrange("p (t f) -> p t f", f=D))

    nc.compile()
    return nc


def _host_tensors(cfg, sched, per_core, edge_index, x, basis, att, root, bias,
                  k_layers=K):
    ns, nsp, tpc = cfg.ns, cfg.nsp, cfg.tpc
    NR = R + 1
    dst = np.asarray(edge_index[1], dtype=np.int64)
    cntp = np.maximum(np.bincount(dst, minlength=cfg.n), 1).astype(np.float32)

    W = np.einsum("krb,kbio->krio", np.asarray(att, np.float32),
                  np.asarray(basis, np.float32))[:k_layers]        # [k,R,D,D]
    Wfull = np.concatenate(
        [W, np.asarray(root, np.float32)[:k_layers, None]], axis=1)  # [k,NR,D,D]
    w_sw = np.ascontiguousarray(
        Wfull.transpose(2, 0, 1, 3).reshape(D, k_layers * NR * D)).astype(BF16)
    bias_in = np.asarray(bias, np.float32)[:k_layers].reshape(
        1, k_layers * D).astype(BF16)
    iota = np.tile(np.arange(128, dtype=np.float32), (128, 1)).astype(BF16)

    x = np.asarray(x, dtype=np.float32)
    in_maps = []
    for c in range(cfg.ncores):
        xp = np.zeros((nsp, D), dtype=np.float32)
        xp[:ns] = x[c * ns:(c + 1) * ns]
        x0 = np.ascontiguousarray(
            xp.reshape(tpc, 128, D).transpose(1, 0, 2).reshape(128, nsp)
        ).astype(BF16)
        cnt_sl = np.zeros(nsp, dtype=np.float32)
        cnt_sl[:ns] = cntp[c * ns:(c + 1) * ns]
        inv_sl = np.ones(nsp, dtype=np.float32)
        inv_sl[:ns] = 1.0 / cntp[c * ns:(c + 1) * ns]
        ar = np.arange(nsp)
        dcs = np.zeros((128, nsp), dtype=np.float32)
        dcs[ar % 128, ar] = cnt_sl
        pc = per_core[c]
        in_maps.append(dict(
            x0=x0, dcs=dcs.astype(BF16),
            cnt_row=cnt_sl.reshape(1, nsp).astype(BF16),
            w_sw=w_sw, bias_in=bias_in,
            invc_in=np.ascontiguousarray(inv_sl.reshape(tpc, 128).T),
            iota_in=iota,
            idx_lo=pc["idx_lo"], cv_lo=pc["cv_lo"],
            idx_hi=pc["idx_hi"], cv_hi=pc["cv_hi"]))
    return in_maps


def _run(cfg, x, edge_index, edge_attr, basis, att, root, bias, prelu_a,
         k_layers=K, trace=False):
    from concourse.bass_utils import run_bass_kernel_spmd

    sched, per_core = _preprocess(cfg, edge_index, edge_attr)
    nc = _build_program(cfg, sched, k_layers,
                        float(np.asarray(prelu_a).ravel()[0]))
    in_maps = _host_tensors(cfg, sched, per_core, edge_index, x, basis, att,
                            root, bias, k_layers)
    res = run_bass_kernel_spmd(nc, in_maps, core_ids=list(range(cfg.ncores)),
                               trace=trace)
    out = np.empty((cfg.n, D), dtype=np.float32)
    for c in range(cfg.ncores):
        out[c * cfg.ns:(c + 1) * cfg.ns] = res.results[c]["out_own"][:cfg.ns]
    return out, res


def kernel(x, edge_index, edge_attr, basis, att, root, bias, prelu_a):
    cfg = Cfg()
    out, _ = _run(cfg, x, edge_index, edge_attr, basis, att, root, bias, prelu_a)
    return out
